# revision 20
# baseline (speedup 1.0000x reference)
"""Single-head attention (B=4, T=4096, D=1024, H=64, fp32 in/out) on 8 TRN2
NeuronCores.

Sharding: one core per (batch, T-half) pair -> 8 shards, no collectives.
Host pre-transposes, pre-casts and pre-packs every input so the device does
zero input transposes, minimal HBM traffic and contiguous-per-partition DMA:
  xt      [8*128, 8*512] bf16  per t-block: [128 part, (d-chunk, 512)] of
                               x[b]^T (query t-blocks first)
  wqt     [128, 8*64]    bf16  Wq^T packed [part, (d-chunk, 64)]
  wkvt    [128, 8*128]   bf16  [Wk^T | Wv^T] packed likewise
  maskt   [4096, 2048]   bf16  mask slice transposed to [s, t]
Each core returns its [2048, 64] f32 slice of the output.

Per-core pipeline:
  P phase (query blocks 0-3): fused k|v + q projections as N=512 matmuls
    accumulating over 8 d-chunks in paired PSUM tiles; PSUM->SBUF copies on
    DVE; V' rows via XBAR dma transposes.
  Attention (j-outer over 32 s-chunks): ST[j] = K[j] qT with qT zero-padded
    to 128 rows and lhsT the full kvT block, so every matmul drives the full
    PE array and the HAM clock gate stays released (2.4 GHz). exp on Act
    ([128,1024] per instruction) is the steady-state pacer; mask multiply on
    DVE (2x bf16); PT fully resident ([128, 32, 2048] bf16). PV matmuls
    (V' zero-padded to 128 columns) trail ST by 2 chunks, emitted adjacent
    so the PE stream stays dense. The k|v projections for the non-query
    x-blocks 4-7 are woven into the first attention iterations.
  Epilogue: 16 PE transposes back to [t, h], reciprocal of the ones-column,
    scale, store.
"""

import sys

if "/opt/trn_rl_repo" not in sys.path:
    sys.path.insert(0, "/opt/trn_rl_repo")

from contextlib import ExitStack

import numpy as np
import ml_dtypes

import concourse.bass as bass
import concourse.tile as tile
from concourse import bacc, mybir
from concourse.bass_utils import run_bass_kernel_spmd
from concourse.masks import make_identity

F32 = mybir.dt.float32
BF16 = mybir.dt.bfloat16

B, T, D, H = 4, 4096, 1024, 64
NCORES = 8
TQ = T // 2  # query rows per core

BF16NP = ml_dtypes.bfloat16


def build_attention_core(T=T, D=D, H=H, Tq=TQ):
    """Build the per-core Bass graph. Every core runs the same graph."""
    assert D % 128 == 0 and T % 1024 == 0 and Tq % 1024 == 0 and H == 64
    DC = D // 128          # d chunks (8)
    NS = T // 128          # s chunks (32)
    NTB = T // 512         # x t-blocks (8)
    NQB = Tq // 512        # query t-blocks (4)
    NSUP = Tq // 512       # t supertiles in attention (4)
    PVLAG = 2              # PV trails ST by this many s-chunks
    scale = 1.0 / float(np.sqrt(D))
    Exp = mybir.ActivationFunctionType.Exp

    nc = bacc.Bacc("TRN2", target_bir_lowering=False, debug=False,
                   num_devices=NCORES)
    xT_ext = nc.declare_dram_parameter("xt", [NTB * 128, DC * 512], BF16,
                                       isOutput=False)
    wqT_ext = nc.declare_dram_parameter("wqt", [128, DC * H], BF16,
                                        isOutput=False)
    wkvT_ext = nc.declare_dram_parameter("wkvt", [128, DC * 2 * H], BF16,
                                         isOutput=False)
    maskT_ext = nc.declare_dram_parameter("maskt", [T, Tq], BF16,
                                          isOutput=False)
    out_ext = nc.declare_dram_parameter("out", [H + 1, Tq], F32,
                                        isOutput=True)

    with tile.TileContext(nc) as tc, ExitStack() as ctx:
        singles = ctx.enter_context(tc.tile_pool(name="singles", bufs=1))
        xin = ctx.enter_context(tc.tile_pool(name="xin", bufs=3))
        mpool = ctx.enter_context(tc.tile_pool(name="mpool", bufs=4))
        opool = ctx.enter_context(tc.tile_pool(name="opool", bufs=1))
        # PSUM: tag "p" [128,2,512] f32 x2 bufs (4 banks) shared by P phase,
        # ST tiles and epilogue; tag "pv" [128,4,512] f32 x1 (4 banks).
        psP = ctx.enter_context(tc.tile_pool(name="psP", bufs=2,
                                             space="PSUM"))
        psV = ctx.enter_context(tc.tile_pool(name="psV", bufs=1,
                                             space="PSUM"))

        ident_bf = singles.tile([128, 128], BF16)
        make_identity(nc, ident_bf)

        # ---- weights (pre-packed on host) ----
        wqT_sb = singles.tile([128, DC, H], BF16)
        nc.scalar.dma_start(
            out=wqT_sb.rearrange("p a b -> p (a b)"), in_=wqT_ext[:, :]
        )
        wkvT_sb = singles.tile([128, DC, 2 * H], BF16)
        nc.scalar.dma_start(
            out=wkvT_sb.rearrange("p a b -> p (a b)"), in_=wkvT_ext[:, :]
        )

        # persistent activations. qT and V' are zero-padded to the full 128
        # partition/column width so attention matmuls light up the whole PE
        # array (HAM un-throttles only under full-array activity): the vT
        # rows of kvT meet zero q rows, and V' columns 65:128 are zero.
        kvT_sb = singles.tile([128, T], BF16)   # rows 0:64 kT, 64:128 vT
        qT_sb = singles.tile([128, Tq], BF16)   # rows 64:128 zero
        Vp_sb = singles.tile([128, NS, 128], BF16)  # V' = [V | 1 | 0pad]
        PT_sb = singles.tile([128, NS, Tq], BF16)   # masked exp scores
        nc.gpsimd.memset(qT_sb[H : 2 * H, :], 0.0)
        nc.gpsimd.memset(Vp_sb[:, :, H + 1 : 128], 0.0)
        nc.gpsimd.memset(Vp_sb[:, :, H : H + 1], 1.0)

        def proj_pair(tbp, with_q):
            """k|v (+q) projections for x t-blocks 2*tbp, 2*tbp+1."""
            kv_ps = psP.tile([128, 2, 512], F32, tag="p", name="kv_ps")
            q_ps = None
            if with_q:
                q_ps = psP.tile([128, 2, 512], F32, tag="p", name="q_ps")
            for half in range(2):
                tb = 2 * tbp + half
                x_sb = xin.tile([128, DC, 512], BF16, tag="x", name="x_sb")
                dma_eng = nc.scalar if tb % 2 == 0 else nc.sync
                dma_eng.dma_start(
                    out=x_sb.rearrange("p a b -> p (a b)"),
                    in_=xT_ext[tb * 128 : (tb + 1) * 128, :],
                )
                for j in range(DC):
                    nc.tensor.matmul(
                        kv_ps[:, half, :],
                        wkvT_sb[:, j, :],
                        x_sb[:, j, :],
                        start=(j == 0),
                        stop=(j == DC - 1),
                    )
                if q_ps is not None:
                    for j in range(DC):
                        nc.tensor.matmul(
                            q_ps[0:H, half, :],
                            wqT_sb[:, j, :],
                            x_sb[:, j, :],
                            start=(j == 0),
                            stop=(j == DC - 1),
                        )
            nc.scalar.copy(
                kvT_sb[:, tbp * 1024 : (tbp + 1) * 1024],
                kv_ps.rearrange("p a b -> p (a b)"),
            )
            if q_ps is not None:
                nc.scalar.copy(
                    qT_sb[0:H, tbp * 1024 : (tbp + 1) * 1024],
                    q_ps[0:H].rearrange("p a b -> p (a b)"),
                )
            # V natural layout for the 8 s-chunks of this t-block pair
            vt_ps = psP.tile([128, 8, H], BF16, tag="p", name="vt_ps")
            for jj in range(8):
                s0 = tbp * 1024 + jj * 128
                nc.tensor.transpose(
                    vt_ps[:, jj, :],
                    kvT_sb[H : 2 * H, s0 : s0 + 128],
                    ident_bf[H : 2 * H, H : 2 * H],
                )
            nc.vector.tensor_copy(
                Vp_sb[:, tbp * 8 : (tbp + 1) * 8, 0:H], vt_ps
            )

        # ---- P phase: query x-blocks only; rest woven into attention ----
        for tbp in range(NQB // 2):
            proj_pair(tbp, with_q=True)

        # ---- attention: ST/exp/mask with PV trailing by PVLAG chunks ----
        pv_ps = psV.tile([128, NSUP, 512], F32, tag="pv")

        def pv_step(j):
            for ts in range(NSUP):
                nc.tensor.matmul(
                    pv_ps[:, ts, :],
                    Vp_sb[:, j, :],
                    PT_sb[:, j, ts * 512 : (ts + 1) * 512],
                    start=(j == 0),
                    stop=(j == NS - 1),
                )

        for j in range(NS):
            if j in (2, 6):
                # weave in the k|v projections for x-block pairs 2 and 3
                proj_pair(2 + (j - 2) // 4, with_q=False)
            m_sb = mpool.tile([128, Tq], BF16, tag="m")
            nc.sync.dma_start(
                out=m_sb, in_=maskT_ext[j * 128 : (j + 1) * 128, :]
            )
            for hh in range(2):
                st_ps = psP.tile([128, 2, 512], F32, tag="p")
                for ts in range(2):
                    t0 = (2 * hh + ts) * 512
                    nc.tensor.matmul(
                        st_ps[:, ts, :],
                        kvT_sb[:, j * 128 : (j + 1) * 128],
                        qT_sb[:, t0 : t0 + 512],
                    )
                nc.scalar.activation(
                    PT_sb[:, j, hh * 1024 : (hh + 1) * 1024],
                    st_ps.rearrange("p a b -> p (a b)"),
                    Exp,
                    scale=scale,
                )
                nc.vector.tensor_mul(
                    PT_sb[:, j, hh * 1024 : (hh + 1) * 1024],
                    PT_sb[:, j, hh * 1024 : (hh + 1) * 1024],
                    m_sb[:, hh * 1024 : (hh + 1) * 1024],
                )
            if j >= PVLAG:
                pv_step(j - PVLAG)
        for j in range(NS - PVLAG, NS):
            pv_step(j)

        # ---- epilogue: ship un-normalized out' (host divides) ----
        oT_sb = opool.tile([H + 1, Tq], F32, tag="oT")
        nc.scalar.copy(
            oT_sb, pv_ps[0 : H + 1].rearrange("p a b -> p (a b)")
        )
        nc.sync.dma_start(out=out_ext[:, :], in_=oT_sb)
    nc.compile()
    return nc


_NC_CACHE = {}


def _get_nc(shape_key):
    if shape_key not in _NC_CACHE:
        T_, D_, H_, Tq_ = shape_key
        _NC_CACHE[shape_key] = build_attention_core(T=T_, D=D_, H=H_, Tq=Tq_)
    return _NC_CACHE[shape_key]


def _pack_dchunks(wt):
    """[D, F] -> [128, DC*F]: partition-major packing of d-chunks."""
    Dv, Fv = wt.shape
    dc = Dv // 128
    return np.ascontiguousarray(
        wt.reshape(dc, 128, Fv).transpose(1, 0, 2).reshape(128, dc * Fv)
    )


def _prep_inputs(x, Wq, Wk, Wv, mask):
    """Host-side shard + transpose + cast + pack. Core c -> (batch c//2,
    half c%2). The x rows of the core's query half come first; mask columns
    get the same permutation so key order matches the permuted x rows."""
    x = np.ascontiguousarray(x, dtype=np.float32)
    mask = np.ascontiguousarray(mask, dtype=np.int32)
    Bv, Tv, Dv = x.shape
    Tq = Tv // 2
    ntb = Tv // 512
    dc = Dv // 128

    wqT = _pack_dchunks(
        np.ascontiguousarray(np.asarray(Wq, dtype=np.float32).T).astype(
            BF16NP
        )
    )
    wkvT = _pack_dchunks(
        np.concatenate(
            [np.asarray(Wk, np.float32).T, np.asarray(Wv, np.float32).T],
            axis=1,
        ).astype(BF16NP)
    )

    def block_xt(xb):
        # [T, D] -> [ (tb, 128part), (d-chunk, 512) ]
        xt = xb.T.astype(BF16NP)  # [D, T]
        x4 = xt.reshape(dc, 128, ntb, 512).transpose(2, 1, 0, 3)
        return np.ascontiguousarray(x4.reshape(ntb * 128, dc * 512))

    # mask is shared across batches: only two variants (one per half)
    m0 = mask[0, 0:Tq, :]  # [t, s] for half 0
    m1 = np.concatenate([mask[0, Tq:, Tq:], mask[0, Tq:, :Tq]], axis=1)
    maskT0 = np.ascontiguousarray(m0.T.astype(BF16NP))
    maskT1 = np.ascontiguousarray(m1.T.astype(BF16NP))

    in_maps = []
    for c in range(NCORES):
        b, half = c // 2, c % 2
        if half == 0:
            xc = x[b]
            mT = maskT0
        else:
            xc = np.concatenate([x[b, Tq:], x[b, :Tq]], axis=0)
            mT = maskT1
        in_maps.append(
            {
                "xt": block_xt(xc),
                "wqt": wqT,
                "wkvt": wkvT,
                "maskt": mT,
            }
        )
    return in_maps


def kernel(x, Wq, Wk, Wv, mask, _trace=False):
    x = np.asarray(x)
    Bv, Tv, Dv = x.shape
    Hv = np.asarray(Wq).shape[0]
    Tq = Tv // 2
    nc = _get_nc((Tv, Dv, Hv, Tq))
    in_maps = _prep_inputs(
        np.asarray(x), np.asarray(Wq), np.asarray(Wk), np.asarray(Wv),
        np.asarray(mask),
    )
    res = run_bass_kernel_spmd(
        nc, in_maps, core_ids=list(range(NCORES)), trace=_trace
    )
    out = np.empty((Bv, Tv, Hv), dtype=np.float32)
    for c in range(NCORES):
        b, half = c // 2, c % 2
        r = res.results[c]["out"]  # [H+1, Tq] un-normalized, transposed
        out[b, half * Tq : (half + 1) * Tq] = (r[0:Hv] / r[Hv : Hv + 1]).T
    if _trace:
        kernel.last_results = res
    return out


# revision 23
# speedup vs baseline: 1.2158x; 1.2158x over previous
"""Single-head attention (B=4, T=4096, D=1024, H=64, fp32 in/out) on 8 TRN2
NeuronCores.

Sharding: one core per (batch, T-half) pair -> 8 shards, no collectives.
Host pre-transposes, pre-casts and pre-packs every input so the device does
zero input transposes, minimal HBM traffic and contiguous-per-partition DMA:
  xt      [8*128, 8*512] bf16  per t-block: [128 part, (d-chunk, 512)] of
                               x[b]^T (query t-blocks first)
  wqt     [128, 8*64]    bf16  Wq^T packed [part, (d-chunk, 64)]
  wkvt    [128, 8*128]   bf16  [Wk^T | Wv^T] packed likewise
  maskt   [4096, 2048]   bf16  mask slice transposed to [s, t]
Each core returns its [2048, 64] f32 slice of the output.

Per-core pipeline:
  P phase (query blocks 0-3): fused k|v + q projections as N=512 matmuls
    accumulating over 8 d-chunks in paired PSUM tiles; PSUM->SBUF copies on
    DVE; V' rows via XBAR dma transposes.
  Attention (j-outer over 32 s-chunks): ST[j] = K[j] qT with qT zero-padded
    to 128 rows and lhsT the full kvT block, so every matmul drives the full
    PE array and the HAM clock gate stays released (2.4 GHz). exp on Act
    ([128,1024] per instruction) is the steady-state pacer; mask multiply on
    DVE (2x bf16); PT fully resident ([128, 32, 2048] bf16). PV matmuls
    (V' zero-padded to 128 columns) trail ST by 2 chunks, emitted adjacent
    so the PE stream stays dense. The k|v projections for the non-query
    x-blocks 4-7 are woven into the first attention iterations.
  Epilogue: 16 PE transposes back to [t, h], reciprocal of the ones-column,
    scale, store.
"""

import sys

if "/opt/trn_rl_repo" not in sys.path:
    sys.path.insert(0, "/opt/trn_rl_repo")

from contextlib import ExitStack

import numpy as np
import ml_dtypes

import concourse.bass as bass
import concourse.tile as tile
from concourse import bacc, mybir
from concourse.bass_utils import run_bass_kernel_spmd
from concourse.masks import make_identity

F32 = mybir.dt.float32
BF16 = mybir.dt.bfloat16

B, T, D, H = 4, 4096, 1024, 64
NCORES = 8
TQ = T // 2  # query rows per core

BF16NP = ml_dtypes.bfloat16


def build_attention_core(T=T, D=D, H=H, Tq=TQ):
    """Build the per-core Bass graph. Every core runs the same graph."""
    assert D % 128 == 0 and T % 1024 == 0 and Tq % 1024 == 0 and H == 64
    DC = D // 128          # d chunks (8)
    NS = T // 128          # s chunks (32)
    NTB = T // 512         # x t-blocks (8)
    NQB = Tq // 512        # query t-blocks (4)
    NSUP = Tq // 512       # t supertiles in attention (4)
    PVLAG = 3              # PV trails ST by this many s-chunks
    scale = 1.0 / float(np.sqrt(D))
    Exp = mybir.ActivationFunctionType.Exp

    nc = bacc.Bacc("TRN2", target_bir_lowering=False, debug=False,
                   num_devices=NCORES)
    xT_ext = nc.declare_dram_parameter("xt", [NTB * 128, DC * 512], BF16,
                                       isOutput=False)
    wqT_ext = nc.declare_dram_parameter("wqt", [128, DC * H], BF16,
                                        isOutput=False)
    wkvT_ext = nc.declare_dram_parameter("wkvt", [128, DC * 2 * H], BF16,
                                         isOutput=False)
    maskT_ext = nc.declare_dram_parameter("maskt", [T, Tq], BF16,
                                          isOutput=False)
    out_ext = nc.declare_dram_parameter("out", [H + 1, Tq], F32,
                                        isOutput=True)

    with tile.TileContext(nc) as tc, ExitStack() as ctx:
        singles = ctx.enter_context(tc.tile_pool(name="singles", bufs=1))
        xin = ctx.enter_context(tc.tile_pool(name="xin", bufs=3))
        mpool = ctx.enter_context(tc.tile_pool(name="mpool", bufs=4))
        opool = ctx.enter_context(tc.tile_pool(name="opool", bufs=1))
        # PSUM: tag "p" [128,2,512] f32 x2 bufs (4 banks) shared by P phase,
        # ST tiles and epilogue; tag "pv" [128,4,512] f32 x1 (4 banks).
        psP = ctx.enter_context(tc.tile_pool(name="psP", bufs=2,
                                             space="PSUM"))
        psV = ctx.enter_context(tc.tile_pool(name="psV", bufs=1,
                                             space="PSUM"))

        ident_bf = singles.tile([128, 128], BF16)
        make_identity(nc, ident_bf)

        # ---- weights (pre-packed on host) ----
        wqT_sb = singles.tile([128, DC, H], BF16)
        nc.scalar.dma_start(
            out=wqT_sb.rearrange("p a b -> p (a b)"), in_=wqT_ext[:, :]
        )
        wkvT_sb = singles.tile([128, DC, 2 * H], BF16)
        nc.scalar.dma_start(
            out=wkvT_sb.rearrange("p a b -> p (a b)"), in_=wkvT_ext[:, :]
        )

        # persistent activations. qT and V' are zero-padded to the full 128
        # partition/column width so attention matmuls light up the whole PE
        # array (HAM un-throttles only under full-array activity): the vT
        # rows of kvT meet zero q rows, and V' columns 65:128 are zero.
        kvT_sb = singles.tile([128, T], BF16)   # rows 0:64 kT, 64:128 vT
        qT_sb = singles.tile([128, Tq], BF16)   # rows 64:128 zero
        Vp_sb = singles.tile([128, NS, 128], BF16)  # V' = [V | 1 | 0pad]
        PT_sb = singles.tile([128, NS, Tq], BF16)   # masked exp scores
        nc.gpsimd.memset(qT_sb[H : 2 * H, :], 0.0)
        nc.gpsimd.memset(Vp_sb[:, :, H + 1 : 128], 0.0)
        nc.gpsimd.memset(Vp_sb[:, :, H : H + 1], 1.0)

        def proj_pair(tbp, with_q):
            """k|v (+q) projections for x t-blocks 2*tbp, 2*tbp+1."""
            kv_ps = psP.tile([128, 2, 512], F32, tag="p", name="kv_ps")
            q_ps = None
            if with_q:
                q_ps = psP.tile([128, 2, 512], F32, tag="p", name="q_ps")
            for half in range(2):
                tb = 2 * tbp + half
                x_sb = xin.tile([128, DC, 512], BF16, tag="x", name="x_sb")
                dma_eng = nc.scalar if tb % 2 == 0 else nc.sync
                dma_eng.dma_start(
                    out=x_sb.rearrange("p a b -> p (a b)"),
                    in_=xT_ext[tb * 128 : (tb + 1) * 128, :],
                )
                for j in range(DC):
                    nc.tensor.matmul(
                        kv_ps[:, half, :],
                        wkvT_sb[:, j, :],
                        x_sb[:, j, :],
                        start=(j == 0),
                        stop=(j == DC - 1),
                    )
                if q_ps is not None:
                    for j in range(DC):
                        nc.tensor.matmul(
                            q_ps[0:H, half, :],
                            wqT_sb[:, j, :],
                            x_sb[:, j, :],
                            start=(j == 0),
                            stop=(j == DC - 1),
                        )
            nc.scalar.copy(
                kvT_sb[:, tbp * 1024 : (tbp + 1) * 1024],
                kv_ps.rearrange("p a b -> p (a b)"),
            )
            if q_ps is not None:
                nc.scalar.copy(
                    qT_sb[0:H, tbp * 1024 : (tbp + 1) * 1024],
                    q_ps[0:H].rearrange("p a b -> p (a b)"),
                )
            # V natural layout for the 8 s-chunks of this t-block pair
            vt_ps = psP.tile([128, 8, H], BF16, tag="p", name="vt_ps")
            for jj in range(8):
                s0 = tbp * 1024 + jj * 128
                nc.tensor.transpose(
                    vt_ps[:, jj, :],
                    kvT_sb[H : 2 * H, s0 : s0 + 128],
                    ident_bf[H : 2 * H, H : 2 * H],
                )
            nc.vector.tensor_copy(
                Vp_sb[:, tbp * 8 : (tbp + 1) * 8, 0:H], vt_ps
            )

        # ---- P phase: query x-blocks only; rest woven into attention ----
        for tbp in range(NQB // 2):
            proj_pair(tbp, with_q=True)

        # ---- attention: ST/exp/mask with PV trailing by PVLAG chunks ----
        pv_ps = psV.tile([128, NSUP, 512], F32, tag="pv")

        def pv_step(j):
            for ts in range(NSUP):
                nc.tensor.matmul(
                    pv_ps[:, ts, :],
                    Vp_sb[:, j, :],
                    PT_sb[:, j, ts * 512 : (ts + 1) * 512],
                    start=(j == 0),
                    stop=(j == NS - 1),
                )

        for j in range(NS):
            if j in (2, 6):
                # weave in the k|v projections for x-block pairs 2 and 3
                proj_pair(2 + (j - 2) // 4, with_q=False)
            m_sb = mpool.tile([128, Tq], BF16, tag="m")
            nc.gpsimd.dma_start(
                out=m_sb, in_=maskT_ext[j * 128 : (j + 1) * 128, :]
            )
            for hh in range(2):
                st_ps = psP.tile([128, 2, 512], F32, tag="p")
                for ts in range(2):
                    t0 = (2 * hh + ts) * 512
                    nc.tensor.matmul(
                        st_ps[:, ts, :],
                        kvT_sb[:, j * 128 : (j + 1) * 128],
                        qT_sb[:, t0 : t0 + 512],
                    )
                nc.scalar.activation(
                    PT_sb[:, j, hh * 1024 : (hh + 1) * 1024],
                    st_ps.rearrange("p a b -> p (a b)"),
                    Exp,
                    scale=scale,
                )
            nc.vector.tensor_mul(
                PT_sb[:, j, :],
                PT_sb[:, j, :],
                m_sb,
            )
            if j >= PVLAG:
                pv_step(j - PVLAG)
        for j in range(NS - PVLAG, NS):
            pv_step(j)

        # ---- epilogue: ship un-normalized out' (host divides) ----
        oT_sb = opool.tile([H + 1, Tq], F32, tag="oT")
        nc.scalar.copy(
            oT_sb, pv_ps[0 : H + 1].rearrange("p a b -> p (a b)")
        )
        nc.sync.dma_start(out=out_ext[:, :], in_=oT_sb)
    nc.compile()
    return nc


_NC_CACHE = {}


def _get_nc(shape_key):
    if shape_key not in _NC_CACHE:
        T_, D_, H_, Tq_ = shape_key
        _NC_CACHE[shape_key] = build_attention_core(T=T_, D=D_, H=H_, Tq=Tq_)
    return _NC_CACHE[shape_key]


def _pack_dchunks(wt):
    """[D, F] -> [128, DC*F]: partition-major packing of d-chunks."""
    Dv, Fv = wt.shape
    dc = Dv // 128
    return np.ascontiguousarray(
        wt.reshape(dc, 128, Fv).transpose(1, 0, 2).reshape(128, dc * Fv)
    )


def _prep_inputs(x, Wq, Wk, Wv, mask):
    """Host-side shard + transpose + cast + pack. Core c -> (batch c//2,
    half c%2). The x rows of the core's query half come first; mask columns
    get the same permutation so key order matches the permuted x rows."""
    x = np.ascontiguousarray(x, dtype=np.float32)
    mask = np.ascontiguousarray(mask, dtype=np.int32)
    Bv, Tv, Dv = x.shape
    Tq = Tv // 2
    ntb = Tv // 512
    dc = Dv // 128

    wqT = _pack_dchunks(
        np.ascontiguousarray(np.asarray(Wq, dtype=np.float32).T).astype(
            BF16NP
        )
    )
    wkvT = _pack_dchunks(
        np.concatenate(
            [np.asarray(Wk, np.float32).T, np.asarray(Wv, np.float32).T],
            axis=1,
        ).astype(BF16NP)
    )

    def block_xt(xb):
        # [T, D] -> [ (tb, 128part), (d-chunk, 512) ]
        xt = xb.T.astype(BF16NP)  # [D, T]
        x4 = xt.reshape(dc, 128, ntb, 512).transpose(2, 1, 0, 3)
        return np.ascontiguousarray(x4.reshape(ntb * 128, dc * 512))

    # mask is shared across batches: only two variants (one per half)
    m0 = mask[0, 0:Tq, :]  # [t, s] for half 0
    m1 = np.concatenate([mask[0, Tq:, Tq:], mask[0, Tq:, :Tq]], axis=1)
    maskT0 = np.ascontiguousarray(m0.T.astype(BF16NP))
    maskT1 = np.ascontiguousarray(m1.T.astype(BF16NP))

    in_maps = []
    for c in range(NCORES):
        b, half = c // 2, c % 2
        if half == 0:
            xc = x[b]
            mT = maskT0
        else:
            xc = np.concatenate([x[b, Tq:], x[b, :Tq]], axis=0)
            mT = maskT1
        in_maps.append(
            {
                "xt": block_xt(xc),
                "wqt": wqT,
                "wkvt": wkvT,
                "maskt": mT,
            }
        )
    return in_maps


def kernel(x, Wq, Wk, Wv, mask, _trace=False):
    x = np.asarray(x)
    Bv, Tv, Dv = x.shape
    Hv = np.asarray(Wq).shape[0]
    Tq = Tv // 2
    nc = _get_nc((Tv, Dv, Hv, Tq))
    in_maps = _prep_inputs(
        np.asarray(x), np.asarray(Wq), np.asarray(Wk), np.asarray(Wv),
        np.asarray(mask),
    )
    res = run_bass_kernel_spmd(
        nc, in_maps, core_ids=list(range(NCORES)), trace=_trace
    )
    out = np.empty((Bv, Tv, Hv), dtype=np.float32)
    for c in range(NCORES):
        b, half = c // 2, c % 2
        r = res.results[c]["out"]  # [H+1, Tq] un-normalized, transposed
        out[b, half * Tq : (half + 1) * Tq] = (r[0:Hv] / r[Hv : Hv + 1]).T
    if _trace:
        kernel.last_results = res
    return out
